# revision 1
# baseline (speedup 1.0000x reference)
"""KNN-Attention Trainium2 kernel.

Sharding: 8 cores = 4 batches x 2 head-groups (8 heads each).
Each core computes a partial output [T, E] = combined_slice @ W_proj_slice;
host sums the two partials per batch.

Per-core device program (SPMD, all per-core variation flows through input data):
  - pass A: qT,kT [1024, T] bf16 matmuls (lhsT=W chunk, rhs=xT)
  - pass B: v natural [T, 512] bf16 (+ ones column for softmax denominators)
  - qnat:   q natural [T, 512] in TRUE fp32 (memory-attention scores need
            exact fp32 dots: softmax scale is E*sqrt(H)=4096, so tiny q errors
            flip near-tied slots vs the reference)
  - mem attention: DVE elementwise in natural layout (exact fp32 scores),
    softmax over K=3 slots, blend with g-prescaled mem_v; PE-transpose into
    combined^T
  - main attention per head: S^T tiles [128tk, 512tq] bf16, exp on ACT
    (scale 1/8, no max subtraction needed: |scores| <~ 3), causal masking via
    precomputed mask tiles on diagonal blocks, AV via lhsT=[v|1] producing
    O'^T [65, 512] with denominators in row 64
  - combine: combT += O'T * (1-g)/denom (partition-broadcast row)
  - c_proj: bf16 matmuls, PSUM -> DRAM partial output
"""

import numpy as np
import ml_dtypes

import concourse.bass as bass
import concourse.mybir as mybir
import concourse.tile as tile
from concourse import bacc
from concourse.bass_utils import run_bass_kernel_spmd
from concourse.masks import make_identity

B, T, E, H, KSLOT = 4, 2048, 1024, 16, 3
D = E // H          # 64
HPC = 8             # heads per core
EC = HPC * D        # 512 cols per core
NCORES = 8
TC = 512            # t-chunk
NCHUNK = T // TC    # 4

f32 = mybir.dt.float32
bf16 = mybir.dt.bfloat16

_CACHE = {}


def _build_nc():
    nc = bacc.Bacc("TRN2", target_bir_lowering=False, debug=False)

    # ---- DRAM I/O ----
    xTf = nc.dram_tensor("xTf", [E, T], f32, kind="ExternalInput").ap()
    xTb = nc.dram_tensor("xTb", [E, T], bf16, kind="ExternalInput").ap()
    wqkv = nc.dram_tensor("wqkv", [E, 3 * EC], bf16, kind="ExternalInput").ap()
    wq32 = nc.dram_tensor("wq32", [E, EC], f32, kind="ExternalInput").ap()
    wp = nc.dram_tensor("wp", [EC, E], bf16, kind="ExternalInput").ap()
    mk = nc.dram_tensor("mk", [T, KSLOT * EC], f32, kind="ExternalInput").ap()
    mvg = nc.dram_tensor("mvg", [T, KSLOT * EC], f32, kind="ExternalInput").ap()
    masks = nc.dram_tensor("masks", [4, 128, TC], bf16, kind="ExternalInput").ap()
    ginv = nc.dram_tensor("ginv", [1, HPC], f32, kind="ExternalInput").ap()
    out = nc.dram_tensor("out", [T, E], f32, kind="ExternalOutput").ap()

    # partition-tiled DRAM views
    xTf_r = xTf.rearrange("(ko p) t -> p ko t", p=128)      # [128, 8, T]
    xTb_r = xTb.rearrange("(ko p) t -> p ko t", p=128)
    wqkv_r = wqkv.rearrange("(ko p) n -> p ko n", p=128)    # [128, 8, 1536]
    wq32_r = wq32.rearrange("(ko p) n -> p ko n", p=128)    # [128, 8, 512]
    wp_r = wp.rearrange("(ko p) n -> p ko n", p=128)        # [128, 4, 1024]

    with tile.TileContext(nc) as tc:
        with (
            tc.tile_pool(name="consts", bufs=1) as consts,
            tc.tile_pool(name="chunk", bufs=2) as chunk,
            tc.tile_pool(name="xtf", bufs=1) as xtfp,
            tc.tile_pool(name="pt", bufs=4) as ptp,
            tc.tile_pool(name="small", bufs=2) as small,
            tc.tile_pool(name="pp", bufs=4, space="PSUM") as pp,
            tc.tile_pool(name="op", bufs=2, space="PSUM") as opp,
            tc.tile_pool(name="tp", bufs=2, space="PSUM") as tpp,
        ):
            # ---- persistent SBUF ----
            wqkv_sb = consts.tile([128, 8, 3 * EC], bf16, tag="wqkv")
            wq_sb = consts.tile([128, 8, EC], f32, tag="wq")
            wp_sb = consts.tile([128, 4, E], bf16, tag="wp")
            masks_sb = consts.tile([128, 4, TC], bf16, tag="masks")
            ginv_sb = consts.tile([1, HPC], f32, tag="ginv")
            ident = consts.tile([128, 128], f32, tag="ident")
            kT_sb = consts.tile([128, 4, T], bf16, tag="kT")
            v_sb = consts.tile([128, T // 128, HPC, D + 1], bf16, tag="v")

            nc.sync.dma_start(wqkv_sb[:], wqkv_r)
            nc.sync.dma_start(wq_sb[:], wq32_r)
            nc.sync.dma_start(wp_sb[:], wp_r)
            nc.sync.dma_start(masks_sb[:], masks.rearrange("m p t -> p m t"))
            nc.sync.dma_start(ginv_sb[:], ginv)
            make_identity(nc, ident[:])
            # ones column for softmax denominators
            nc.vector.memset(v_sb[:, :, :, D], 1.0)

            for c in range(NCHUNK):
                ts = slice(c * TC, (c + 1) * TC)
                xtb_c = chunk.tile([128, 8, TC], bf16, tag="xtb")
                nc.sync.dma_start(xtb_c[:], xTb_r[:, :, ts])
                xtf_c = xtfp.tile([128, 8, TC], f32, tag="xtf")
                nc.sync.dma_start(xtf_c[:], xTf_r[:, :, ts])

                qT_c = chunk.tile([128, 4, TC], bf16, tag="qT")
                combT_c = chunk.tile([128, 4, TC], bf16, tag="combT")

                # ---- pass A: qT (m 0-3) and kT (m 4-7), bf16 ----
                for m in range(8):
                    ps = pp.tile([128, TC], f32, tag="pp512")
                    for ke in range(8):
                        nc.tensor.matmul(
                            ps[:],
                            wqkv_sb[:, ke, 128 * m : 128 * (m + 1)],
                            xtb_c[:, ke, :],
                            start=(ke == 0),
                            stop=(ke == 7),
                        )
                    if m < 4:
                        nc.vector.tensor_copy(qT_c[:, m, :], ps[:])
                    else:
                        nc.vector.tensor_copy(kT_sb[:, m - 4, ts], ps[:])

                # ---- pass B: v natural [TC, 512] bf16 ----
                for tb in range(4):
                    ps = pp.tile([128, TC], f32, tag="pp512")
                    for ke in range(8):
                        nc.tensor.matmul(
                            ps[:],
                            xtb_c[:, ke, 128 * tb : 128 * (tb + 1)],
                            wqkv_sb[:, ke, 2 * EC : 3 * EC],
                            start=(ke == 0),
                            stop=(ke == 7),
                        )
                    for h in range(HPC):
                        nc.vector.tensor_copy(
                            v_sb[:, 4 * c + tb, h, 0:D], ps[:, D * h : D * (h + 1)]
                        )

                # ---- qnat (true fp32) + memory attention per t-block ----
                for tb in range(4):
                    trow = slice(c * TC + 128 * tb, c * TC + 128 * (tb + 1))
                    ps = pp.tile([128, TC], f32, tag="pp512")
                    for ke in range(8):
                        nc.tensor.matmul(
                            ps[:],
                            xtf_c[:, ke, 128 * tb : 128 * (tb + 1)],
                            wq_sb[:, ke, :],
                            start=(ke == 0),
                            stop=(ke == 7),
                        )
                    qn = chunk.tile([128, EC], f32, tag="qn")
                    nc.vector.tensor_copy(qn[:], ps[:])

                    mk_t = chunk.tile([128, KSLOT, EC], f32, tag="mk")
                    nc.sync.dma_start(
                        mk_t[:], mk[trow, :].rearrange("p (k e) -> p k e", k=KSLOT)
                    )
                    mv_t = chunk.tile([128, KSLOT, EC], f32, tag="mv")
                    nc.sync.dma_start(
                        mv_t[:], mvg[trow, :].rearrange("p (k e) -> p k e", k=KSLOT)
                    )

                    # exact fp32 scores: s3[t, k, h] = sum_d qn*mk
                    prod = chunk.tile([128, KSLOT, EC], f32, tag="prod")
                    nc.vector.tensor_mul(
                        prod[:], mk_t[:], qn[:, None, :].to_broadcast((128, KSLOT, EC))
                    )
                    s3 = small.tile([128, KSLOT, HPC], f32, tag="s3")
                    nc.vector.tensor_reduce(
                        s3[:],
                        prod[:].rearrange("p k (h d) -> p k h d", d=D),
                        mybir.AxisListType.X,
                        mybir.AluOpType.add,
                    )
                    m3 = small.tile([128, HPC], f32, tag="m3")
                    nc.vector.tensor_reduce(
                        m3[:],
                        s3[:].rearrange("p k h -> p h k"),
                        mybir.AxisListType.X,
                        mybir.AluOpType.max,
                    )
                    z3 = small.tile([128, KSLOT, HPC], f32, tag="z3")
                    nc.vector.tensor_sub(
                        z3[:], s3[:], m3[:, None, :].to_broadcast((128, KSLOT, HPC))
                    )
                    e3 = small.tile([128, KSLOT, HPC], f32, tag="e3")
                    nc.scalar.activation(
                        e3[:], z3[:], mybir.ActivationFunctionType.Exp,
                        scale=float(E) * float(np.sqrt(H)),
                    )
                    den = small.tile([128, HPC], f32, tag="den")
                    nc.vector.tensor_reduce(
                        den[:],
                        e3[:].rearrange("p k h -> p h k"),
                        mybir.AxisListType.X,
                        mybir.AluOpType.add,
                    )
                    rden = small.tile([128, HPC], f32, tag="rden")
                    nc.vector.reciprocal(rden[:], den[:])
                    w3 = small.tile([128, KSLOT, HPC], f32, tag="w3")
                    nc.vector.tensor_mul(
                        w3[:], e3[:], rden[:, None, :].to_broadcast((128, KSLOT, HPC))
                    )
                    # blend: mm[t, e] = sum_k w3[t,k,h(e)] * mvg[t,k,e]
                    wprod = chunk.tile([128, KSLOT, EC], f32, tag="prod")
                    nc.vector.tensor_mul(
                        wprod[:].rearrange("p k (h d) -> p k h d", d=D),
                        mv_t[:].rearrange("p k (h d) -> p k h d", d=D),
                        w3[:, :, :, None].to_broadcast((128, KSLOT, HPC, D)),
                    )
                    mm_t = chunk.tile([128, EC], f32, tag="mm")
                    nc.vector.tensor_add(mm_t[:], wprod[:, 0, :], wprod[:, 1, :])
                    nc.vector.tensor_add(mm_t[:], mm_t[:], wprod[:, 2, :])

                    # transpose mem output into combT (combT = g*mem part; g
                    # was folded into mvg on host)
                    for ec in range(4):
                        tps = tpp.tile([128, 128], f32, tag="tp")
                        nc.tensor.transpose(
                            tps[:], mm_t[:, 128 * ec : 128 * (ec + 1)], ident[:]
                        )
                        nc.scalar.activation(
                            combT_c[:, ec, 128 * tb : 128 * (tb + 1)], tps[:],
                            mybir.ActivationFunctionType.Copy,
                        )

                # ---- main attention per head ----
                for h in range(HPC):
                    prow = slice(64 * (h % 2), 64 * (h % 2) + 64)
                    pc = h // 2
                    ops = opp.tile([65, TC], f32, tag="ops")
                    njt = 4 * c + 4
                    for j in range(njt):
                        sps = pp.tile([128, TC], f32, tag="pp512")
                        nc.tensor.matmul(
                            sps[:],
                            kT_sb[prow, pc, 128 * j : 128 * (j + 1)],
                            qT_c[prow, pc, :],
                            start=True,
                            stop=True,
                        )
                        pt = ptp.tile([128, TC], bf16, tag="pt")
                        nc.scalar.activation(
                            pt[:], sps[:], mybir.ActivationFunctionType.Exp,
                            scale=1.0 / np.sqrt(D),
                        )
                        if j >= 4 * c:
                            nc.vector.tensor_mul(
                                pt[:], pt[:], masks_sb[:, j - 4 * c, :]
                            )
                        nc.tensor.matmul(
                            ops[:],
                            v_sb[:, j, h, :],
                            pt[:],
                            start=(j == 0),
                            stop=(j == njt - 1),
                        )
                    # normalize + accumulate into combT
                    rr = small.tile([1, TC], f32, tag="rr")
                    nc.vector.reciprocal(rr[:], ops[64:65, :])
                    rr2 = small.tile([1, TC], f32, tag="rr2")
                    nc.vector.tensor_scalar_mul(rr2[:], rr[:], ginv_sb[0:1, h : h + 1])
                    bc = small.tile([128, TC], f32, tag="bc")
                    nc.gpsimd.partition_broadcast(bc[:], rr2[0:1, :])
                    tmp = small.tile([128, TC], f32, tag="tmpo")
                    nc.vector.tensor_mul(tmp[prow, :], ops[0:64, :], bc[prow, :])
                    nc.vector.tensor_add(
                        combT_c[prow, pc, :], combT_c[prow, pc, :], tmp[prow, :]
                    )

                # ---- c_proj partial: out[tc-rows, :] ----
                for tb in range(4):
                    trow = slice(c * TC + 128 * tb, c * TC + 128 * (tb + 1))
                    for n in range(2):
                        ps = pp.tile([128, TC], f32, tag="pp512")
                        for ke in range(4):
                            nc.tensor.matmul(
                                ps[:],
                                combT_c[:, ke, 128 * tb : 128 * (tb + 1)],
                                wp_sb[:, ke, TC * n : TC * (n + 1)],
                                start=(ke == 0),
                                stop=(ke == 3),
                            )
                        ost = chunk.tile([128, TC], f32, tag="ost")
                        nc.vector.tensor_copy(ost[:], ps[:])
                        nc.sync.dma_start(out[trow, TC * n : TC * (n + 1)], ost[:])

    nc.compile()
    return nc


def _prep_inputs(x, mem_k, mem_v, W_attn, W_proj, gate_bias):
    """Build per-core input maps (host-side sharding/layout only)."""
    in_maps = []
    g = gate_bias.reshape(H)
    for core in range(NCORES):
        b, hg = core // 2, core % 2
        cs = slice(hg * EC, (hg + 1) * EC)
        xb = np.asarray(x[b], dtype=np.float32)            # [T, E]
        xT = np.ascontiguousarray(xb.T)                    # [E, T]
        wq = np.ascontiguousarray(W_attn[:, cs])           # [E, 512]
        wk = np.ascontiguousarray(W_attn[:, E + hg * EC : E + (hg + 1) * EC])
        wv = np.ascontiguousarray(W_attn[:, 2 * E + hg * EC : 2 * E + (hg + 1) * EC])
        wqkv = np.concatenate([wq, wk, wv], axis=1)        # [E, 1536]
        gh = g[hg * HPC : (hg + 1) * HPC].astype(np.float32)   # [8]
        mkc = np.ascontiguousarray(mem_k[b][:, :, cs]).reshape(T, KSLOT * EC)
        mvc = np.ascontiguousarray(mem_v[b][:, :, cs]).astype(np.float32)
        # fold gate into mem_v: combined = mem*g + y*(1-g)
        mvc = mvc * gh.repeat(D)[None, None, :]
        mvc = mvc.reshape(T, KSLOT * EC)
        wpc = np.ascontiguousarray(W_proj[cs, :])          # [512, E]
        # causal mask tiles for the 4 diagonal sub-blocks
        m4 = np.zeros((4, 128, TC), dtype=np.float32)
        tkh = np.arange(128)[:, None]
        tqh = np.arange(TC)[None, :]
        for mi in range(4):
            m4[mi] = (128 * mi + tkh <= tqh).astype(np.float32)
        in_maps.append(
            {
                "xTf": xT,
                "xTb": xT.astype(ml_dtypes.bfloat16),
                "wqkv": wqkv.astype(ml_dtypes.bfloat16),
                "wq32": wq,
                "wp": wpc.astype(ml_dtypes.bfloat16),
                "mk": mkc.astype(np.float32),
                "mvg": mvc.astype(np.float32),
                "masks": m4.astype(ml_dtypes.bfloat16),
                "ginv": (1.0 - gh).reshape(1, HPC),
            }
        )
    return in_maps


def kernel(x, mem_k, mem_v, W_attn, W_proj, gate_bias, **kw):
    x = np.asarray(x, dtype=np.float32)
    mem_k = np.asarray(mem_k, dtype=np.float32)
    mem_v = np.asarray(mem_v, dtype=np.float32)
    W_attn = np.asarray(W_attn, dtype=np.float32)
    W_proj = np.asarray(W_proj, dtype=np.float32)
    gate_bias = np.asarray(gate_bias, dtype=np.float32)

    if "nc" not in _CACHE:
        _CACHE["nc"] = _build_nc()
    nc = _CACHE["nc"]
    in_maps = _prep_inputs(x, mem_k, mem_v, W_attn, W_proj, gate_bias)
    res = run_bass_kernel_spmd(nc, in_maps, list(range(NCORES)), **kw)
    results = res.results if hasattr(res, "results") else res
    out = np.empty((B, T, E), dtype=np.float32)
    for b in range(B):
        out[b] = results[2 * b]["out"] + results[2 * b + 1]["out"]
    _CACHE["last_res"] = res
    return out



# revision 8
# speedup vs baseline: 1.1211x; 1.1211x over previous
"""KNN-Attention Trainium2 kernel.

Sharding: 8 cores = 4 batches x 2 head-groups (8 heads each).
Each core computes a partial output [T, E] = combined_slice @ W_proj_slice;
host sums the two partials per batch.

Per-core device program (SPMD, all per-core variation flows through input data):
  - qnat:  q natural [T, 512] via fp32r matmuls (1 cyc/row on PE, full fp32
           precision; memory-attention scores need exact fp32 dots since the
           softmax scale is E*sqrt(H)=4096)
  - qT:    PE-transpose of qnat (f32), copied to bf16 for main attention
  - kT:    [512, T] fp32r matmuls, copied to bf16
  - v:     natural [T, 512] fp32r matmuls -> bf16 (+ ones column for softmax
           denominators); (1-g) is folded into W_v on host
  - mem attention: DVE elementwise in natural layout (exact fp32 scores),
    softmax over K=3 slots; the big value-blend ops run on the idle Pool
    (gpsimd) engine; mem_v is bf16 with g prescaled (values don't affect
    slot selection, only scores do)
  - main attention per head: S^T tiles [128tk, <=512tq] bf16 with causal
    narrowing (free dim starts at the diagonal), exp on ACT (scale 1/8),
    triangular [128,128] mask only on diagonal blocks; AV flipped:
    lhsT=pt block [128k,128q], moving=v [128k,65] -> y natural [128q, 65]
    with denominators in col 64 (65 rows/matmul instead of 512)
  - combine: scalar_tensor_tensor fuses y*(1/den) + mem into comb bf16
    (per-partition scalar broadcast; no gpsimd partition_broadcast needed)
  - comb -> combT via bf16 PE transposes; c_proj bf16 matmuls -> DRAM
"""

import numpy as np
import ml_dtypes

import concourse.bass as bass
import concourse.mybir as mybir
import concourse.tile as tile
from concourse import bacc
from concourse.bass_utils import run_bass_kernel_spmd
from concourse.masks import make_identity

B, T, E, H, KSLOT = 4, 2048, 1024, 16, 3
D = E // H          # 64
HPC = 8             # heads per core
EC = HPC * D        # 512 cols per core
NCORES = 8
TC = 512            # t-chunk
NCHUNK = T // TC    # 4

f32 = mybir.dt.float32
f32r = mybir.dt.float32r
bf16 = mybir.dt.bfloat16

_CACHE = {}


def _build_nc():
    nc = bacc.Bacc("TRN2", target_bir_lowering=False, debug=False)

    # ---- DRAM I/O ----
    xTf = nc.dram_tensor("xTf", [E, T], f32r, kind="ExternalInput").ap()
    wqkv = nc.dram_tensor("wqkv", [E, 3 * EC], f32r, kind="ExternalInput").ap()
    wp = nc.dram_tensor("wp", [EC, E], bf16, kind="ExternalInput").ap()
    mk = nc.dram_tensor("mk", [T, KSLOT * EC], f32, kind="ExternalInput").ap()
    mvg = nc.dram_tensor("mvg", [T, KSLOT * EC], bf16, kind="ExternalInput").ap()
    tri = nc.dram_tensor("tri", [128, 128], bf16, kind="ExternalInput").ap()
    out = nc.dram_tensor("out", [T, E], f32, kind="ExternalOutput").ap()

    # partition-tiled DRAM views
    xTf_r = xTf.rearrange("(ko p) t -> p ko t", p=128)      # [128, 8, T]
    wqkv_r = wqkv.rearrange("(ko p) n -> p ko n", p=128)    # [128, 8, 1536]
    wp_r = wp.rearrange("(ko p) n -> p ko n", p=128)        # [128, 4, 1024]

    mem_scale = float(E) * float(np.sqrt(H))

    with tile.TileContext(nc) as tc:
        with (
            tc.tile_pool(name="consts", bufs=1) as consts,
            tc.tile_pool(name="xtf", bufs=2) as xtfp,
            tc.tile_pool(name="qn", bufs=1) as qnp,
            tc.tile_pool(name="qt", bufs=2) as qtp,
            tc.tile_pool(name="ptl", bufs=1) as ptp,
            tc.tile_pool(name="comb", bufs=1) as combp,
            tc.tile_pool(name="mem", bufs=2) as memp,
            tc.tile_pool(name="mm", bufs=1) as mmp,
            tc.tile_pool(name="small", bufs=2) as small,
            tc.tile_pool(name="ost", bufs=1) as ostp,
            tc.tile_pool(name="pp", bufs=2, space="PSUM") as pp,
            tc.tile_pool(name="sps", bufs=2, space="PSUM") as spsp,
            tc.tile_pool(name="yp", bufs=2, space="PSUM") as ypp,
            tc.tile_pool(name="tpb", bufs=2, space="PSUM") as tpbp,
        ):
            # ---- persistent SBUF ----
            wqkv_sb = consts.tile([128, 8, 3 * EC], f32r, tag="wqkv")
            wp_sb = consts.tile([128, 4, E], bf16, tag="wp")
            tri_sb = consts.tile([128, 128], bf16, tag="tri")
            identf = consts.tile([128, 128], f32, tag="identf")
            identb = consts.tile([128, 128], bf16, tag="identb")
            kT_sb = consts.tile([128, 4, T], bf16, tag="kT")
            v_sb = consts.tile([128, T // 128, HPC, D + 1], bf16, tag="v")

            for ke in range(8):
                nc.sync.dma_start(wqkv_sb[:, ke, :], wqkv_r[:, ke, :])
            nc.sync.dma_start(wp_sb[:], wp_r)
            nc.sync.dma_start(tri_sb[:], tri)
            make_identity(nc, identf[:])
            nc.vector.tensor_copy(identb[:], identf[:])
            # ones column for softmax denominators
            nc.vector.memset(v_sb[:, :, :, D], 1.0)

            for c in range(NCHUNK):
                ts = slice(c * TC, (c + 1) * TC)
                xtf_c = xtfp.tile([128, 8, TC], f32r, tag="xtf")
                nc.sync.dma_start(xtf_c[:], xTf_r[:, :, ts])

                # ---- qnat: q natural [tb*128, 512] in fp32r ----
                qn_c = qnp.tile([128, 4, TC], f32, tag="qn")
                for tb in range(4):
                    ps = pp.tile([128, TC], f32, tag="pp512")
                    for ke in range(8):
                        nc.tensor.matmul(
                            ps[:],
                            xtf_c[:, ke, 128 * tb : 128 * (tb + 1)],
                            wqkv_sb[:, ke, 0:EC],
                            start=(ke == 0),
                            stop=(ke == 7),
                        )
                    nc.vector.tensor_copy(qn_c[:, tb, :], ps[:])

                # ---- mem attention per t-block (DVE small ops + Pool blends) ----
                mm_c = mmp.tile([128, 4, EC], f32, tag="mm")
                for tb in range(4):
                    trow = slice(c * TC + 128 * tb, c * TC + 128 * (tb + 1))
                    mk_t = memp.tile([128, KSLOT, EC], f32, tag="mk")
                    nc.sync.dma_start(
                        mk_t[:], mk[trow, :].rearrange("p (k e) -> p k e", k=KSLOT)
                    )
                    mv_t = memp.tile([128, KSLOT, EC], bf16, tag="mv")
                    nc.sync.dma_start(
                        mv_t[:], mvg[trow, :].rearrange("p (k e) -> p k e", k=KSLOT)
                    )

                    # exact fp32 scores: s3[t, k, h] = sum_d qn*mk
                    prod = memp.tile([128, KSLOT, EC], f32, tag="prod")
                    nc.vector.tensor_mul(
                        prod[:],
                        mk_t[:],
                        qn_c[:, tb, None, :].to_broadcast((128, KSLOT, EC)),
                    )
                    s3 = small.tile([128, KSLOT, HPC], f32, tag="s3")
                    nc.vector.tensor_reduce(
                        s3[:],
                        prod[:].rearrange("p k (h d) -> p k h d", d=D),
                        mybir.AxisListType.X,
                        mybir.AluOpType.add,
                    )
                    m3 = small.tile([128, HPC], f32, tag="m3")
                    nc.vector.tensor_reduce(
                        m3[:],
                        s3[:].rearrange("p k h -> p h k"),
                        mybir.AxisListType.X,
                        mybir.AluOpType.max,
                    )
                    z3 = small.tile([128, KSLOT, HPC], f32, tag="z3")
                    nc.vector.tensor_sub(
                        z3[:], s3[:], m3[:, None, :].to_broadcast((128, KSLOT, HPC))
                    )
                    e3 = small.tile([128, KSLOT, HPC], f32, tag="e3")
                    nc.scalar.activation(
                        e3[:], z3[:], mybir.ActivationFunctionType.Exp,
                        scale=mem_scale,
                    )
                    den = small.tile([128, HPC], f32, tag="den")
                    nc.vector.tensor_reduce(
                        den[:],
                        e3[:].rearrange("p k h -> p h k"),
                        mybir.AxisListType.X,
                        mybir.AluOpType.add,
                    )
                    rden = small.tile([128, HPC], f32, tag="rden")
                    nc.vector.reciprocal(rden[:], den[:])
                    w3 = small.tile([128, KSLOT, HPC], f32, tag="w3")
                    nc.vector.tensor_mul(
                        w3[:], e3[:], rden[:, None, :].to_broadcast((128, KSLOT, HPC))
                    )
                    # blend (Pool engine): mm[t, e] = sum_k w3[t,k,h(e)] * mvg[t,k,e]
                    wprod = memp.tile([128, KSLOT, EC], f32, tag="prod")
                    nc.gpsimd.tensor_mul(
                        wprod[:].rearrange("p k (h d) -> p k h d", d=D),
                        mv_t[:].rearrange("p k (h d) -> p k h d", d=D),
                        w3[:, :, :, None].to_broadcast((128, KSLOT, HPC, D)),
                    )
                    nc.gpsimd.tensor_add(
                        mm_c[:, tb, :], wprod[:, 0, :], wprod[:, 1, :]
                    )
                    nc.gpsimd.tensor_add(
                        mm_c[:, tb, :], mm_c[:, tb, :], wprod[:, 2, :]
                    )

                # ---- qT: PE-transpose qnat (f32), copy to bf16 ----
                qT_c = qtp.tile([128, 4, TC], bf16, tag="qT")
                for fb in range(4):
                    tp = pp.tile([128, TC], f32, tag="pp512")
                    for tb in range(4):
                        nc.tensor.transpose(
                            tp[:, 128 * tb : 128 * (tb + 1)],
                            qn_c[:, tb, 128 * fb : 128 * (fb + 1)],
                            identf[:],
                        )
                    nc.vector.tensor_copy(qT_c[:, fb, :], tp[:])

                # ---- kT: [512, T] fp32r, transposed layout ----
                for m in range(4):
                    ps = pp.tile([128, TC], f32, tag="pp512")
                    for ke in range(8):
                        nc.tensor.matmul(
                            ps[:],
                            wqkv_sb[:, ke, EC + 128 * m : EC + 128 * (m + 1)],
                            xtf_c[:, ke, :],
                            start=(ke == 0),
                            stop=(ke == 7),
                        )
                    nc.vector.tensor_copy(kT_sb[:, m, ts], ps[:])

                # ---- v natural [tb*128, 512] fp32r ----
                for tb in range(4):
                    ps = pp.tile([128, TC], f32, tag="pp512")
                    for ke in range(8):
                        nc.tensor.matmul(
                            ps[:],
                            xtf_c[:, ke, 128 * tb : 128 * (tb + 1)],
                            wqkv_sb[:, ke, 2 * EC : 3 * EC],
                            start=(ke == 0),
                            stop=(ke == 7),
                        )
                    nc.vector.tensor_copy(
                        v_sb[:, 4 * c + tb, :, 0:D],
                        ps[:].rearrange("p (h d) -> p h d", d=D),
                    )

                # ---- main attention per head ----
                njt = 4 * c + 4
                pt_c = ptp.tile([128, 16, TC], bf16, tag="pt")
                comb_c = combp.tile([128, 4, EC], bf16, tag="comb")
                for h in range(HPC):
                    prow = slice(64 * (h % 2), 64 * (h % 2) + 64)
                    pc = h // 2
                    # phase 1: scores + exp (+ triangle mask on diagonal)
                    for j in range(njt):
                        off = 128 * (j - 4 * c) if j >= 4 * c else 0
                        sps = spsp.tile([128, TC], f32, tag="sps")
                        nc.tensor.matmul(
                            sps[:, off:TC],
                            kT_sb[prow, pc, 128 * j : 128 * (j + 1)],
                            qT_c[prow, pc, off:TC],
                            start=True,
                            stop=True,
                        )
                        nc.scalar.activation(
                            pt_c[:, j, off:TC], sps[:, off:TC],
                            mybir.ActivationFunctionType.Exp,
                            scale=1.0 / float(np.sqrt(D)),
                        )
                        if j >= 4 * c:
                            nc.vector.tensor_mul(
                                pt_c[:, j, off : off + 128],
                                pt_c[:, j, off : off + 128],
                                tri_sb[:],
                            )
                    # phase 2: AV flipped -> y natural [128q, 65] per qtile
                    y_h = ypp.tile([128, TC], f32, tag="y")
                    for r in range(4):
                        nj = 4 * c + r + 1
                        for j in range(nj):
                            nc.tensor.matmul(
                                y_h[:, 128 * r : 128 * r + D + 1],
                                pt_c[:, j, 128 * r : 128 * (r + 1)],
                                v_sb[:, j, h, :],
                                start=(j == 0),
                                stop=(j == nj - 1),
                            )
                    # normalize + combine with mem output
                    rg = small.tile([128, 4], f32, tag="rg")
                    nc.vector.reciprocal(
                        rg[:], y_h[:].rearrange("p (r x) -> p r x", x=128)[:, :, D]
                    )
                    for r in range(4):
                        nc.vector.scalar_tensor_tensor(
                            comb_c[:, r, D * h : D * (h + 1)],
                            y_h[:, 128 * r : 128 * r + D],
                            rg[:, r : r + 1],
                            mm_c[:, r, D * h : D * (h + 1)],
                            mybir.AluOpType.mult,
                            mybir.AluOpType.add,
                        )

                # ---- comb -> combT (bf16 PE transposes) ----
                combT_c = qtp.tile([128, 4, EC], bf16, tag="combT")
                for fb in range(4):
                    tpb = tpbp.tile([128, TC], bf16, tag="tpb")
                    for r in range(4):
                        nc.tensor.transpose(
                            tpb[:, 128 * r : 128 * (r + 1)],
                            comb_c[:, r, 128 * fb : 128 * (fb + 1)],
                            identb[:],
                        )
                    nc.vector.tensor_copy(combT_c[:, fb, :], tpb[:])

                # ---- c_proj partial: out[tc-rows, :] ----
                for tb in range(4):
                    trow = slice(c * TC + 128 * tb, c * TC + 128 * (tb + 1))
                    for n in range(2):
                        ps = pp.tile([128, TC], f32, tag="pp512")
                        for ke in range(4):
                            nc.tensor.matmul(
                                ps[:],
                                combT_c[:, ke, 128 * tb : 128 * (tb + 1)],
                                wp_sb[:, ke, TC * n : TC * (n + 1)],
                                start=(ke == 0),
                                stop=(ke == 3),
                            )
                        ost = ostp.tile([128, TC], f32, tag="ost")
                        nc.scalar.copy(ost[:], ps[:])
                        nc.sync.dma_start(out[trow, TC * n : TC * (n + 1)], ost[:])

    nc.compile()
    return nc


def _prep_inputs(x, mem_k, mem_v, W_attn, W_proj, gate_bias):
    """Build per-core input maps (host-side sharding/layout only)."""
    in_maps = []
    g = gate_bias.reshape(H)
    tri = np.triu(np.ones((128, 128), dtype=np.float32))
    for core in range(NCORES):
        b, hg = core // 2, core % 2
        cs = slice(hg * EC, (hg + 1) * EC)
        xb = np.asarray(x[b], dtype=np.float32)            # [T, E]
        xT = np.ascontiguousarray(xb.T)                    # [E, T]
        gh = g[hg * HPC : (hg + 1) * HPC].astype(np.float32)   # [8]
        wq = np.ascontiguousarray(W_attn[:, cs])           # [E, 512]
        wk = np.ascontiguousarray(W_attn[:, E + hg * EC : E + (hg + 1) * EC])
        wv = np.ascontiguousarray(W_attn[:, 2 * E + hg * EC : 2 * E + (hg + 1) * EC])
        # fold (1-g) into W_v: y uses v*(1-g)
        wv = wv * (1.0 - gh).repeat(D)[None, :]
        wqkv = np.concatenate([wq, wk, wv], axis=1)        # [E, 1536]
        mkc = np.ascontiguousarray(mem_k[b][:, :, cs]).reshape(T, KSLOT * EC)
        mvc = np.ascontiguousarray(mem_v[b][:, :, cs]).astype(np.float32)
        # fold gate into mem_v: combined = mem*g + y*(1-g)
        mvc = mvc * gh.repeat(D)[None, None, :]
        mvc = mvc.reshape(T, KSLOT * EC)
        wpc = np.ascontiguousarray(W_proj[cs, :])          # [512, E]
        in_maps.append(
            {
                "xTf": xT,
                "wqkv": wqkv.astype(np.float32),
                "wp": wpc.astype(ml_dtypes.bfloat16),
                "mk": mkc.astype(np.float32),
                "mvg": mvc.astype(ml_dtypes.bfloat16),
                "tri": tri.astype(ml_dtypes.bfloat16),
            }
        )
    return in_maps


def kernel(x, mem_k, mem_v, W_attn, W_proj, gate_bias, **kw):
    x = np.asarray(x, dtype=np.float32)
    mem_k = np.asarray(mem_k, dtype=np.float32)
    mem_v = np.asarray(mem_v, dtype=np.float32)
    W_attn = np.asarray(W_attn, dtype=np.float32)
    W_proj = np.asarray(W_proj, dtype=np.float32)
    gate_bias = np.asarray(gate_bias, dtype=np.float32)

    if "nc" not in _CACHE:
        _CACHE["nc"] = _build_nc()
    nc = _CACHE["nc"]
    in_maps = _prep_inputs(x, mem_k, mem_v, W_attn, W_proj, gate_bias)
    res = run_bass_kernel_spmd(nc, in_maps, list(range(NCORES)), **kw)
    results = res.results if hasattr(res, "results") else res
    out = np.empty((B, T, E), dtype=np.float32)
    for b in range(B):
        out[b] = results[2 * b]["out"] + results[2 * b + 1]["out"]
    _CACHE["last_res"] = res
    return out
